# revision 18
# baseline (speedup 1.0000x reference)
"""Trainium2 Bass kernel for nn_NodeAttention (hypergraph message passing).

Math (reference):
    w      = sigmoid(x @ attn_w.T + attn_b)[:, 0]          # per-edge weight (M == N)
    e_feat = Binv * segsum_by_edge(xl[node_idx]),  xl = x @ lin_w.T
    D      = segsum_by_node(w[edge_idx])
    out    = Dinv * segsum_by_node([e_feat|w][edge_idx])[:, :C] + bias

Structure: 3 SPMD launches on 8 cores with host data-movement between them
(sharding / concat / row replication only — every arithmetic op on x-derived
data runs on device).

  P0: core c computes xl and w for its 1/8 slab of x (fp16 matmul with the
      attention column fused as a 129th output channel).
  A:  node->edge. The host replicates xl rows into an entry-ordered,
      partition-major fp16 stream (np.take; entries grouped by destination
      edge window). The device streams it sequentially (no gathers) and
      segment-sums each 128-edge window with one-hot matmuls, scales by
      Binv, and emits ea rows [e_feat(128) | w | pad3] fp16.
  B:  edge->node. Same pattern over ea rows; Dinv = 1/max(D, tiny) with the
      D column falling out of the same matmul; + bias.

Per-window destinations are load-balanced across (core, window) bins by an
LPT assignment on host (pure index work), so every window has the same
static tile count and SPMD cores run identical programs.

Rationale: dma_gather costs ~8.6 ns/row in Q7 descriptor generation
(measured) regardless of row size — 100k rows/core/phase = ~860 us/phase
floor. Sequential streams avoid per-row descriptors entirely.
"""

import heapq
import os
import sys
from contextlib import ExitStack

import numpy as np

try:
    import ml_dtypes
    BF16_NP = ml_dtypes.bfloat16
except ImportError:  # pragma: no cover
    BF16_NP = None

for _p in (
    "/root/.axon_site",
    "/root/.axon_site/_ro/trn_rl_repo",
    "/root/.axon_site/_ro/pypackages",
):
    if os.path.isdir(_p) and _p not in sys.path:
        sys.path.append(_p)

import concourse.bass as bass  # noqa: E402
import concourse.mybir as mybir  # noqa: E402
import concourse.tile as tile  # noqa: E402
from concourse import bacc  # noqa: E402
from concourse.bass_utils import run_bass_kernel_spmd  # noqa: E402

P = 128
N_NODES = 50000
N_EDGES = 50000
C = 128
CB = 132          # ea row: [e_feat(128) | w | pad3] fp16
NCORES = 8
WPC = 49          # windows per core
NBINS = NCORES * WPC          # 392 destination bins of 128 lanes
SLOTS = WPC * P               # 6272 slots per core
TOTSLOTS = NCORES * SLOTS     # 50176 >= 50000

F16 = mybir.dt.bfloat16  # bf16: DVE 16-bit fast path is bf16-only
F32 = mybir.dt.float32

TRACE = False
LAST_EXEC_NS = {}

_PROGRAMS = {}


# ----------------------------------------------------------------------------
# Host-side slot assignment and entry images (index work only)
# ----------------------------------------------------------------------------

def _assign_slots(weights):
    """LPT assignment of items to NBINS bins of capacity P, minimizing the
    max bin weight. Returns slot id per item (bin*P + lane)."""
    n = weights.shape[0]
    order = np.argsort(-weights, kind="stable")
    heap = [(0, b) for b in range(NBINS)]
    heapq.heapify(heap)
    counts = np.zeros(NBINS, np.int64)
    bin_of = np.empty(n, np.int64)
    for idx in order:
        while True:
            wsum, b = heapq.heappop(heap)
            if counts[b] < P:
                break
        bin_of[idx] = b
        counts[b] += 1
        heapq.heappush(heap, (wsum + int(weights[idx]), b))
    # lane = arrival rank within bin (by descending weight; irrelevant)
    order2 = np.argsort(bin_of, kind="stable")
    starts = np.cumsum(np.bincount(bin_of, minlength=NBINS)) - np.bincount(
        bin_of, minlength=NBINS
    )
    rank = np.empty(n, np.int64)
    rank[order2] = np.arange(n) - starts[bin_of[order2]]
    return bin_of * P + rank


def _entry_images(dst_slot, src_idx):
    """Group entries by destination bin; lay them out in tiles of 128.

    Returns (t, dst_img [NCORES,P,WPC*t] f16 with -1 pads,
             idx_img [NCORES,P,WPC*t] int64 with 0 pads).
    Entry at rank r of bin (core,w) sits at partition r%P, column w*t + r//P.
    """
    bins = dst_slot // P
    lanes = (dst_slot % P).astype(BF16_NP)
    order = np.argsort(bins, kind="stable")
    b_sorted = bins[order]
    counts = np.bincount(bins, minlength=NBINS)
    t = int(np.ceil(counts.max() / P))
    starts = np.cumsum(counts) - counts
    rank = np.arange(dst_slot.shape[0]) - starts[b_sorted]
    core = b_sorted // WPC
    win = b_sorted - core * WPC
    tl = rank // P
    pp = rank - tl * P
    col = win * t + tl
    dst_img = np.full((NCORES, P, WPC * t), -1.0, BF16_NP)
    idx_img = np.zeros((NCORES, P, WPC * t), np.int64)
    dst_img[core, pp, col] = lanes[order]
    idx_img[core, pp, col] = src_idx[order]
    return t, dst_img, idx_img


# ----------------------------------------------------------------------------
# Bass programs
# ----------------------------------------------------------------------------

def _new_nc():
    return bacc.Bacc(
        "TRN2",
        target_bir_lowering=False,
        debug=False,
        enable_asserts=False,
        num_devices=NCORES,
    )


def _p0_program():
    """Per-core slab: xl = x @ lin_w.T (fp16) and w = sigmoid(x.a + b)."""
    nc = _new_nc()
    xT = nc.dram_tensor("xT", [P, SLOTS], F32, kind="ExternalInput").ap()
    wa = nc.dram_tensor("wa", [P, CB], F32, kind="ExternalInput").ap()
    bcol = nc.dram_tensor("bcol", [P, 1], F32, kind="ExternalInput").ap()
    xl = nc.dram_tensor("xl", [SLOTS, C], F16, kind="ExternalOutput").ap()
    wvec = nc.dram_tensor("wvec", [P, WPC], F32, kind="ExternalOutput").ap()

    with tile.TileContext(nc) as tc:
        with ExitStack() as ctx:
            const = ctx.enter_context(tc.tile_pool(name="const", bufs=1))
            xpool = ctx.enter_context(tc.tile_pool(name="x", bufs=3))
            wpool = ctx.enter_context(tc.tile_pool(name="wk", bufs=3))
            opool = ctx.enter_context(tc.tile_pool(name="out", bufs=3))
            pps = ctx.enter_context(tc.tile_pool(name="ps", bufs=4, space="PSUM"))

            wa32 = const.tile([P, CB], F32)
            nc.sync.dma_start(out=wa32[:], in_=wa[:])
            wa16 = const.tile([P, CB], F16)
            nc.scalar.copy(wa16[:], wa32[:])
            b_sb = const.tile([P, 1], F32)
            nc.sync.dma_start(out=b_sb[:], in_=bcol[:])
            w_all = const.tile([P, WPC], F32)

            GRP = 4  # tiles per DMA / conversion batch
            ngrp = (WPC + GRP - 1) // GRP
            for gi in range(ngrp):
                t0 = gi * GRP
                nt = min(GRP, WPC - t0)
                xt32 = xpool.tile([P, GRP * P], F32, tag="x32")
                nc.sync.dma_start(
                    out=xt32[:, : nt * P],
                    in_=xT[:, t0 * P : (t0 + nt) * P],
                )
                xt16 = wpool.tile([P, GRP * P], F16, tag="x16")
                nc.vector.tensor_copy(xt16[:, : nt * P], xt32[:, : nt * P])
                xl_g = opool.tile([P, GRP * C], F16, tag="xl")
                for k in range(nt):
                    tt = t0 + k
                    ps = pps.tile([P, CB], F32)
                    nc.tensor.matmul(
                        out=ps[:], lhsT=xt16[:, k * P : (k + 1) * P],
                        rhs=wa16[:], start=True, stop=True,
                    )
                    nc.scalar.copy(xl_g[:, k * C : (k + 1) * C], ps[:, 0:C])
                    nc.scalar.activation(
                        w_all[:, tt : tt + 1], ps[:, C : C + 1],
                        mybir.ActivationFunctionType.Sigmoid,
                        bias=b_sb[:, 0:1], scale=1.0,
                    )
                nc.sync.dma_start(
                    out=xl[t0 * P : (t0 + nt) * P, :].rearrange(
                        "(t p) c -> p t c", p=P
                    ),
                    in_=xl_g[:, : nt * C].rearrange("p (t c) -> p t c", c=C),
                )
            nc.sync.dma_start(out=wvec[:], in_=w_all[:])
    nc.compile()
    return nc


def _phase_program(phase, t):
    """Streamed segment-sum phase. phase 'A': rhs width C, Binv scale + w
    column, f16 out rows [C+4]. phase 'B': rhs width CB, Dinv scale + bias,
    f32 out rows [C]."""
    cw = C if phase == "A" else CB
    nc = _new_nc()
    stream = nc.dram_tensor(
        "stream", [P, WPC * t * cw], F16, kind="ExternalInput"
    ).ap()
    dst = nc.dram_tensor("dst", [P, WPC * t], F16, kind="ExternalInput").ap()
    if phase == "A":
        binv = nc.dram_tensor("binv", [P, WPC], F32, kind="ExternalInput").ap()
        wimg = nc.dram_tensor("wimg", [P, WPC], F32, kind="ExternalInput").ap()
        eout = nc.dram_tensor("eout", [SLOTS, CB], F16, kind="ExternalOutput").ap()
    else:
        biasr = nc.dram_tensor("biasr", [P, C], F32, kind="ExternalInput").ap()
        oout = nc.dram_tensor("oout", [SLOTS, C], F32, kind="ExternalOutput").ap()

    CHUNK = 2  # windows per stream DMA
    with tile.TileContext(nc) as tc:
        with ExitStack() as ctx:
            const = ctx.enter_context(tc.tile_pool(name="const", bufs=1))
            gpool = ctx.enter_context(tc.tile_pool(name="g", bufs=3))
            spool = ctx.enter_context(tc.tile_pool(name="s", bufs=3))
            wpool = ctx.enter_context(tc.tile_pool(name="wk", bufs=3))
            opool = ctx.enter_context(tc.tile_pool(name="o", bufs=3))
            pps = ctx.enter_context(tc.tile_pool(name="ps", bufs=4, space="PSUM"))

            iota_i = const.tile([P, P], mybir.dt.int32)
            nc.gpsimd.iota(iota_i[:], pattern=[[1, P]], base=0, channel_multiplier=0)
            iota_f = const.tile([P, P], F16)
            nc.vector.tensor_copy(iota_f[:], iota_i[:])
            # full-width iota [P, t*P] so the is_equal has one plain operand
            iota_big = const.tile([P, t * P], F16)
            nc.vector.tensor_copy(
                iota_big[:].rearrange("p (t c) -> p t c", c=P),
                iota_f[:]
                .rearrange("p (one c) -> p one c", one=1)
                .to_broadcast([P, t, P]),
            )
            dst_sb = const.tile([P, WPC * t], F16)
            nc.sync.dma_start(out=dst_sb[:], in_=dst[:])
            if phase == "A":
                binv_sb = const.tile([P, WPC], F32)
                nc.sync.dma_start(out=binv_sb[:], in_=binv[:])
                w32 = const.tile([P, WPC], F32)
                nc.sync.dma_start(out=w32[:], in_=wimg[:])
            else:
                bias_sb = const.tile([P, C], F32)
                nc.sync.dma_start(out=bias_sb[:], in_=biasr[:])

            nwchunks = (WPC + CHUNK - 1) // CHUNK
            for ck in range(nwchunks):
                w0 = ck * CHUNK
                nw = min(CHUNK, WPC - w0)
                g = gpool.tile([P, CHUNK * t * cw], F16, tag="g")
                nc.sync.dma_start(
                    out=g[:, : nw * t * cw],
                    in_=stream[:, w0 * t * cw : (w0 + nw) * t * cw],
                )
                for wi in range(nw):
                    w = w0 + wi
                    goff = wi * t * cw
                    s_all = spool.tile([P, t * P], F16, tag="s")
                    nc.vector.tensor_tensor(
                        s_all[:].rearrange("p (t c) -> p t c", c=P),
                        dst_sb[:, w * t : (w + 1) * t]
                        .rearrange("p (t one) -> p t one", one=1)
                        .to_broadcast([P, t, P]),
                        iota_big[:].rearrange("p (t c) -> p t c", c=P),
                        op=mybir.AluOpType.is_equal,
                    )
                    ps = pps.tile([P, cw], F32)
                    for k in range(t):
                        nc.tensor.matmul(
                            out=ps[:],
                            lhsT=s_all[:, k * P : (k + 1) * P],
                            rhs=g[:, goff + k * cw : goff + (k + 1) * cw],
                            start=(k == 0),
                            stop=(k == t - 1),
                        )
                    if phase == "A":
                        ot = opool.tile([P, CB], F16, tag="ot")
                        nc.scalar.activation(
                            ot[:, 0:C], ps[:, 0:C],
                            mybir.ActivationFunctionType.Copy,
                            scale=binv_sb[:, w : w + 1],
                        )
                        nc.scalar.copy(ot[:, C : C + 1], w32[:, w : w + 1])
                        nc.vector.memset(ot[:, C + 1 : CB], 0.0)
                        nc.sync.dma_start(
                            out=eout[w * P : (w + 1) * P, :], in_=ot[:]
                        )
                    else:
                        dmax = wpool.tile([P, 1], F32, tag="dmax")
                        nc.vector.tensor_scalar_max(dmax[:], ps[:, C : C + 1], 1e-30)
                        dinv = wpool.tile([P, 1], F32, tag="dinv")
                        nc.vector.reciprocal(dinv[:], dmax[:])
                        ot = opool.tile([P, C], F32, tag="ot")
                        nc.scalar.activation(
                            ot[:], ps[:, 0:C],
                            mybir.ActivationFunctionType.Copy,
                            scale=dinv[:, 0:1],
                        )
                        nc.vector.tensor_tensor(
                            ot[:], ot[:], bias_sb[:], op=mybir.AluOpType.add
                        )
                        nc.sync.dma_start(
                            out=oout[w * P : (w + 1) * P, :], in_=ot[:]
                        )
    nc.compile()
    return nc


def _program(key, builder):
    if key not in _PROGRAMS:
        _PROGRAMS[key] = builder()
    return _PROGRAMS[key]


def _run(nc, in_maps, label):
    kwargs = {}
    if TRACE:
        kwargs = dict(trace=True, trace_cores=[0])
    res = run_bass_kernel_spmd(nc, in_maps, core_ids=list(range(NCORES)), **kwargs)
    if res.exec_time_ns is not None:
        LAST_EXEC_NS[label] = res.exec_time_ns
    return res.results


# ----------------------------------------------------------------------------
# Entry point
# ----------------------------------------------------------------------------

def kernel(x, hyperedge_index, attn_w, attn_b, lin_w, bias):
    x = np.ascontiguousarray(np.asarray(x, dtype=np.float32))
    he = np.asarray(hyperedge_index)
    node_idx = he[0].astype(np.int64)
    edge_idx = he[1].astype(np.int64)
    attn_w = np.asarray(attn_w, dtype=np.float32).reshape(-1)
    attn_b = float(np.asarray(attn_b, dtype=np.float32).reshape(-1)[0])
    lin_w = np.asarray(lin_w, dtype=np.float32)
    bias = np.asarray(bias, dtype=np.float32).reshape(-1)

    # ---- host index preprocessing -----------------------------------------
    bdeg = np.bincount(edge_idx, minlength=N_EDGES)
    ddeg = np.bincount(node_idx, minlength=N_NODES)
    slotE = _assign_slots(bdeg)          # edge -> ea-table slot
    slotN = _assign_slots(ddeg)          # node -> output slot
    tA, dstA, idxA = _entry_images(slotE[edge_idx], node_idx)
    tB, dstB, idxB = _entry_images(slotN[node_idx], slotE[edge_idx])

    binv_full = np.zeros(TOTSLOTS, np.float32)
    binv_full[slotE] = np.where(bdeg > 0, 1.0 / np.maximum(bdeg, 1), 0.0)
    # [core][p, w] layouts (slot = core*SLOTS + w*P + p)
    binv_img = binv_full.reshape(NCORES, WPC, P).transpose(0, 2, 1).copy()

    wa = np.zeros((P, CB), np.float32)
    wa[:, :C] = lin_w.T
    wa[:, C] = attn_w
    b_col = np.full((P, 1), attn_b, np.float32)
    bias_rep = np.ascontiguousarray(np.broadcast_to(bias, (P, C)))

    # P0 x slab, transposed, padded to SLOTS rows
    x_slabT = np.zeros((NCORES, P, SLOTS), np.float32)
    per = N_NODES // NCORES
    for c in range(NCORES):
        rows = x[c * per : (c + 1) * per]
        x_slabT[c, :, : rows.shape[0]] = rows.T

    # ---- P0: xl + w --------------------------------------------------------
    nc0 = _program("P0", _p0_program)
    in0 = [
        {"xT": x_slabT[c], "wa": wa, "bcol": b_col} for c in range(NCORES)
    ]
    res0 = _run(nc0, in0, "P0")
    xl_full = np.concatenate(
        [r["xl"][:per] for r in res0], axis=0
    )  # [50000, 128] f16
    w_full = np.concatenate(
        [r["wvec"].T.reshape(-1)[:per] for r in res0], axis=0
    )  # [50000] f32

    w_slot = np.zeros(TOTSLOTS, np.float32)
    w_slot[slotE] = w_full
    w_img = w_slot.reshape(NCORES, WPC, P).transpose(0, 2, 1).copy()

    # ---- phase A: node -> edge --------------------------------------------
    ncA = _program(("A", tA), lambda: _phase_program("A", tA))
    inA = []
    for c in range(NCORES):
        streamA = xl_full[idxA[c]].reshape(P, -1)  # [P, WPC*tA*C] f16
        inA.append(
            {
                "stream": streamA,
                "dst": dstA[c],
                "binv": binv_img[c],
                "wimg": w_img[c],
            }
        )
    resA = _run(ncA, inA, "A")
    ea = np.concatenate([r["eout"] for r in resA], axis=0)  # [TOTSLOTS, 132] f16

    # ---- phase B: edge -> node --------------------------------------------
    ncB = _program(("B", tB), lambda: _phase_program("B", tB))
    inB = []
    for c in range(NCORES):
        streamB = ea[idxB[c]].reshape(P, -1)  # [P, WPC*tB*CB] f16
        inB.append({"stream": streamB, "dst": dstB[c], "biasr": bias_rep})
    resB = _run(ncB, inB, "B")
    out_slots = np.concatenate([r["oout"] for r in resB], axis=0)  # [TOTSLOTS, C]

    out = out_slots[slotN]
    return np.ascontiguousarray(out.astype(np.float32))
